# revision 8
# baseline (speedup 1.0000x reference)
"""Trainium2 Bass kernel for nn_ConvexReLU.

Math: out[i,m] = sum_{j,k,l} G[j,k] * x[i,k,l] * (v-w)[j,l,m]

Reassociated as:
    d = v - w                              (host, elementwise)
    T[k,l,m]   = sum_j G[j,k] * d[j,l,m]   (device matmul, 68.7 GFLOP)
    out[i,m]   = sum_{k,l} x[i,k,l] * T[k,l,m]   (device matmul, 17.2 GFLOP)

Sharding: split l (in_dim, 256) across 8 cores (32 each). Each core computes
a full-shape (out_dim, batch) partial; host sums the 8 partials.

Device layout per core:
    g  : (1024 j, 1024 k)        full G, replicated; 2KB/partition descriptors
    d  : (4 pair, 8 jc, 128 p, 1024) l-shard of v-w, pg-pair-major so each
         DMA descriptor is 2KB contiguous; fully prefetched (bufs=4)
    xt : (32 l, 128 p, 8 kt, 256 i) l-shard of x, partition-major so each
         descriptor is 4KB contiguous
    out: (128 m, 256 i)          partial of out^T

Head optimizations vs v1: tiny first DMA bites (g[:,0:128] + d[:,0:256]) with
column-split first matmuls so the PE starts as soon as ~96KB lands; dummy
warmup matmuls on a memset tile burn the PE pstate ramp during the DMA-feed
latency window; d fully prefetched so stage-1 never waits on d past pg0.
Tail: final PSUM->SBUF copy + DRAM DMA split in halves across both rings.
"""

import os
import sys

import numpy as np

for _p in ("/opt/trn_rl_repo", "/root/.axon_site/_ro/trn_rl_repo"):
    if os.path.isdir(_p) and _p not in sys.path:
        sys.path.insert(0, _p)

import concourse.bass as bass
import concourse.bacc as bacc
import concourse.mybir as mybir
from concourse.bass_utils import run_bass_kernel_spmd
from concourse.tile import TileContext

B, J, K, L, M = 256, 1024, 1024, 256, 128
NCORES = 8
LC = L // NCORES          # 32 l-values per core
NPG = 8                   # l-groups per core
LG = LC // NPG            # 4 l-values per group
NKT = K // 128            # 8 k-tiles
NJC = J // 128            # 8 j-chunks
NPAIR = NPG // 2          # pg-pairs for d tiles

F32 = mybir.dt.float32
F32R = mybir.dt.float32r
BF16 = mybir.dt.bfloat16

DTYPE = os.environ.get("BASS_KERNEL_DTYPE", "bf16")
N_WARM = int(os.environ.get("BASS_N_WARM", "56"))


def _dtypes(dtype_name: str):
    if dtype_name == "bf16":
        return BF16, BF16
    if dtype_name == "mixed":
        return F32R, BF16
    return F32R, F32R


def build_nc(dtype_name: str = DTYPE) -> bass.Bass:
    gd_dt, s2_dt = _dtypes(dtype_name)

    nc = bacc.Bacc(None, debug=False)

    g = nc.declare_dram_parameter("g", [J, K], gd_dt, isOutput=False)
    # d: (pair, jc, p, pair_cols) so each partition row is 2KB contiguous
    d = nc.declare_dram_parameter(
        "d", [NPAIR, NJC, 128, 2 * LG * M], gd_dt, isOutput=False
    )
    # xt: (l, p, kt*i) so each partition row is 4KB contiguous
    xt = nc.declare_dram_parameter("xt", [LC, 128, NKT * B], s2_dt, isOutput=False)
    out = nc.declare_dram_parameter("out", [M, B], F32, isOutput=True)

    g_r = g.rearrange("(jc p) k -> p jc k", p=128)
    d_r = d.rearrange("t jc p f -> t p jc f")
    xt_r = xt.rearrange("l p (kt i) -> l p kt i", kt=NKT)
    PW = 2 * LG * M  # 1024: columns per pg-pair in a d tile

    with TileContext(nc) as tc:
        with (
            tc.tile_pool(name="wpool", bufs=1) as wpool,
            tc.tile_pool(name="gpool", bufs=1) as gpool,
            tc.tile_pool(name="dpool", bufs=4) as dpool,
            tc.tile_pool(name="tpool", bufs=3) as tpool,
            tc.tile_pool(name="xpool", bufs=8) as xpool,
            tc.tile_pool(name="opool", bufs=1) as opool,
            tc.tile_pool(name="ps1", bufs=6, space="PSUM") as ps1,
            tc.tile_pool(name="pso", bufs=1, space="PSUM") as pso,
            tc.tile_pool(name="psw", bufs=1, space="PSUM") as psw,
        ):
            # ---- PE warmup: burn the DVFS pstate ramp on dummy matmuls
            # while the first real operands are still in DMA flight.
            warm = wpool.tile([128, 32], gd_dt)
            nc.gpsimd.memset(warm[:], 0)
            warm_ps = psw.tile([32, 32], F32)
            for _ in range(N_WARM):
                nc.tensor.matmul(
                    warm_ps[:], warm[:], warm[:],
                    start=True, stop=True, skip_group_check=True,
                )

            # ---- head DMAs: alternate rings per jc chunk; first bites tiny
            # so the first matmul's operands land as early as possible.
            g_sb = gpool.tile([128, NJC, K], gd_dt)
            d_tiles = []
            d_sb0 = dpool.tile([128, NJC, PW], gd_dt, tag="d", name="d_p0")
            d_tiles.append(d_sb0)

            # first bites, split by partition halves so the first matmuls
            # need only ~100KB: a = partitions 0:64, b = 64:128
            nc.sync.dma_start(out=g_sb[0:64, 0, 0:768], in_=g_r[0:64, 0, 0:768])
            nc.scalar.dma_start(out=d_sb0[0:64, 0, 0:512], in_=d_r[0, 0:64, 0, 0:512])
            nc.sync.dma_start(out=g_sb[64:128, 0, 0:768], in_=g_r[64:128, 0, 0:768])
            nc.scalar.dma_start(
                out=d_sb0[64:128, 0, 0:512], in_=d_r[0, 64:128, 0, 0:512]
            )
            nc.sync.dma_start(out=g_sb[:, 0, 768:1024], in_=g_r[:, 0, 768:1024])
            nc.scalar.dma_start(out=d_sb0[:, 0, 512:1024], in_=d_r[0, :, 0, 512:1024])
            for jc in range(1, NJC):
                ga = nc.sync if jc % 2 == 0 else nc.scalar
                da = nc.scalar if jc % 2 == 0 else nc.sync
                ga.dma_start(out=g_sb[:, jc, :], in_=g_r[:, jc, :])
                da.dma_start(out=d_sb0[:, jc, :], in_=d_r[0, :, jc, :])
            # d pairs 1..3 get one coalesced DMA each (2KB descriptors) but
            # are issued inside the pg loop, after the preceding stage-2's x
            # DMAs, so they don't clog the ring feed ahead of x.
            for t in range(1, NPAIR):
                d_tiles.append(
                    dpool.tile([128, NJC, PW], gd_dt, tag="d", name=f"d_p{t}")
                )

            out_ps = pso.tile([M, B], F32)

            total_mm2 = NPG * LG * NKT
            # kt-groups per stage-1 pass: (6,2) so each jc chunk yields 6
            # back-to-back matmuls early on; psum: 6 stage-1 + 1 out + 1 warm
            KGROUPS = [(0, 6), (6, 2)]
            KH = 4  # stage-2 kt-group width

            mm2_state = [0]

            def stage2(pg, t_sb):
                # out^T += T^T-slices @ x^T-slices for l-group pg.
                xs = []
                for dl in range(LG):
                    x_sb = xpool.tile(
                        [128, NKT, B], s2_dt, tag="x", name=f"x_{pg}_{dl}"
                    )
                    ring = nc.sync if dl % 2 == 0 else nc.scalar
                    ring.dma_start(out=x_sb[:], in_=xt_r[pg * LG + dl])
                    xs.append(x_sb)
                for half in range(NKT // KH):
                    for dl in range(LG):
                        for kt2 in range(KH):
                            kt = half * KH + kt2
                            nc.tensor.matmul(
                                out_ps[:],
                                t_sb[:, kt, dl * M : (dl + 1) * M],
                                xs[dl][:, kt, :],
                                start=(mm2_state[0] == 0),
                                stop=(mm2_state[0] == total_mm2 - 1),
                                skip_group_check=True,
                            )
                            mm2_state[0] += 1

            prev = None  # (pg, t_sb) whose stage-2 is pending

            for pg in range(NPG):
                d_sb = d_tiles[pg // 2]
                dc0 = (pg % 2) * LG * M      # column offset of this pg in pair
                dc1 = dc0 + LG * M

                t_sb = tpool.tile([128, NKT, LG * M], s2_dt, tag="t")
                for gi, (k0, kn) in enumerate(KGROUPS):
                    p1s = [
                        ps1.tile([128, LG * M], F32, tag="p1",
                                 name=f"p1_{pg}_{gi}_{i}")
                        for i in range(kn)
                    ]
                    if pg == 0 and gi == 0:
                        # jc0 split by partition halves: the a-chain needs
                        # only the ~100KB first DMA bites, so the PE starts
                        # ~1us earlier. Full columns each => exactly one
                        # start=True per PSUM tile.
                        for half, (p0, p1) in enumerate(((0, 64), (64, 128))):
                            for kt2 in range(kn):
                                kt = k0 + kt2
                                nc.tensor.matmul(
                                    p1s[kt2][:],
                                    g_sb[p0:p1, 0, kt * 128 : (kt + 1) * 128],
                                    d_sb[p0:p1, 0, dc0:dc1],
                                    start=(half == 0),
                                    stop=False,
                                    skip_group_check=True,
                                )
                        jc_range = range(1, NJC)
                    else:
                        jc_range = range(NJC)
                    for jc in jc_range:
                        for kt2 in range(kn):
                            kt = k0 + kt2
                            nc.tensor.matmul(
                                p1s[kt2][:],
                                g_sb[:, jc, kt * 128 : (kt + 1) * 128],
                                d_sb[:, jc, dc0:dc1],
                                start=(jc == 0),
                                stop=(jc == NJC - 1),
                                skip_group_check=True,
                            )
                    for kt2 in range(kn):
                        kt = k0 + kt2
                        nc.vector.tensor_copy(out=t_sb[:, kt, :], in_=p1s[kt2][:])

                # stage-2 lags stage-1 by one l-group
                if prev is not None:
                    stage2(*prev)
                prev = (pg, t_sb)
                # d pair t is needed by pg=2t; issue its DMA behind the x
                # DMAs of stage2(pg-1) so x isn't starved on the rings
                if pg % 2 == 1 and pg // 2 + 1 < NPAIR:
                    t = pg // 2 + 1
                    ring = nc.sync if t % 2 == 1 else nc.scalar
                    ring.dma_start(out=d_tiles[t][:], in_=d_r[t])

            stage2(*prev)

            out_sb = opool.tile([M, B], F32)
            nc.vector.tensor_copy(out=out_sb[:, 0:128], in_=out_ps[:, 0:128])
            nc.sync.dma_start(out=out[:, 0:128], in_=out_sb[:, 0:128])
            nc.vector.tensor_copy(out=out_sb[:, 128:256], in_=out_ps[:, 128:256])
            nc.scalar.dma_start(out=out[:, 128:256], in_=out_sb[:, 128:256])

    nc.finalize()
    return nc


_NC_CACHE: dict[str, bass.Bass] = {}


def _get_nc(dtype_name: str = DTYPE) -> bass.Bass:
    if dtype_name not in _NC_CACHE:
        _NC_CACHE[dtype_name] = build_nc(dtype_name)
    return _NC_CACHE[dtype_name]


def make_in_maps(x, G, v, w, dtype_name: str = DTYPE):
    x = np.asarray(x, dtype=np.float32)
    G = np.asarray(G, dtype=np.float32)
    v = np.asarray(v, dtype=np.float32)
    w = np.asarray(w, dtype=np.float32)

    d_full = v - w  # (J, L, M)

    import ml_dtypes

    if dtype_name == "bf16":
        gd_np, x_np = ml_dtypes.bfloat16, ml_dtypes.bfloat16
    elif dtype_name == "mixed":
        gd_np, x_np = np.float32, ml_dtypes.bfloat16
    else:
        gd_np, x_np = np.float32, np.float32

    G_io = np.ascontiguousarray(G.astype(gd_np))
    in_maps = []
    for c in range(NCORES):
        ls = slice(c * LC, (c + 1) * LC)
        # d (J, LC, M) -> (pair, jc, p, 2*LG*M): pair-major, 2KB rows
        d_c = d_full[:, ls, :].reshape(NJC, 128, NPAIR, 2 * LG, M)
        d_c = np.ascontiguousarray(
            d_c.transpose(2, 0, 1, 3, 4).reshape(NPAIR, NJC, 128, 2 * LG * M)
            .astype(gd_np)
        )
        # x (B, K, L) -> xt (LC, p, kt*i): partition-major, 4KB rows
        xt_c = x[:, :, ls].transpose(2, 1, 0).reshape(LC, NKT, 128, B)
        xt_c = np.ascontiguousarray(
            xt_c.transpose(0, 2, 1, 3).reshape(LC, 128, NKT * B).astype(x_np)
        )
        in_maps.append({"g": G_io, "d": d_c, "xt": xt_c})
    return in_maps


def kernel(x, G, v, w):
    nc = _get_nc()
    in_maps = make_in_maps(x, G, v, w)
    res = run_bass_kernel_spmd(nc, in_maps, core_ids=list(range(NCORES)))
    acc = np.zeros((M, B), dtype=np.float64)
    for r in res.results:
        acc += r["out"].astype(np.float64)
    return np.ascontiguousarray(acc.T.astype(np.float32))
